# revision 1
# baseline (speedup 1.0000x reference)
"""Chamfer 2D loss kernel for Trainium2 (8 NeuronCores, SPMD).

Problem: N=16 objects, two point sets [16, 4096, 2] fp32 each.
Per object: C[i,j] = sqrt(clip(|x_i|^2 + |y_j|^2 - 2 x_i.y_j, 1e-12));
loss = mean_n mask_n * 0.5 * (mean_i min_j C + mean_j min_i C).

Sharding: data-parallel over objects, 2 objects per core; per core 4
"generations" (2 objects x 2 directions), each computing the 4096
per-query min squared distances for query set Q vs key set K.

Per generation the j-space (keys) is split between two exact-fp32 pipelines
that run concurrently on different engines:
  - j in [0, 2048): TensorE K=3 fp32 matmuls (4 concurrent via row-group
    tile_position packing - K=3 uses 3 of 128 PE rows, so 4 independent
    matmuls share the array) computing out[i,j] = q_i.k_j - |k_j|^2/2 into
    PSUM; ScalarE drains each PSUM tile to SBUF; a custom DVE op folds the
    two halves with max and max-reduces rows (2 keys/cycle);
    min_j dist^2 = |q_i|^2 - 2*max_j out.
  - j in [2048, 4096): keys replicated across all 128 partitions; a second
    custom DVE op computes min_j (k0-q0)^2 + (k1-q1)^2 directly (queries as
    per-partition scalars), 1 j/cycle, no PE/ACT involvement.
min over j of sqrt == sqrt of min over j (monotonic), so only the 4096
per-query minima need sqrt, done on host with the means and the mask.
"""

import contextlib

import numpy as np

import concourse.bacc as bacc
import concourse.bass as bass
import concourse.tile as tile
from concourse import mybir
import concourse.dve_ops as dve_ops
from concourse.dve_ops import DveOp
from concourse.dve_spec import (
    Spec, Src0, Src1, C0, C1, C2, maxx, minn, sq, lower, _has_src1,
)
from concourse.dve_uop import DveOpSpec
from concourse.bass_utils import run_bass_kernel_spmd

F32 = mybir.dt.float32
ALU = mybir.AluOpType

N_CORES = 8
N_OBJ = 16
P = 4096          # points per set
OBJ_PER_CORE = N_OBJ // N_CORES      # 2
N_GENS = 2 * OBJ_PER_CORE            # 4 generations per core
IT = P // 128                        # 32 i-tiles of 128 queries
JP = 2048                            # keys handled by the PE pipeline
JD = P - JP                          # keys handled by the DVE-direct pipeline
EPS = 1e-12
NEG_BIG = -3.0e38
POS_BIG = 3.0e38


def _maxmax_ref(in0, in1, s0, s1, imm2):
    b = np.maximum(in0, in1).astype(np.float32)
    acc = np.maximum(
        np.asarray(s0, np.float32),
        b.reshape(b.shape[0], -1).max(axis=-1, keepdims=True),
    ).astype(np.float32)
    return b, acc


def _distmin_ref(in0, in1, s0, s1, imm2):
    b = ((in0 - np.asarray(s0, np.float32)) ** 2
         + (in1 - np.asarray(s1, np.float32)) ** 2).astype(np.float32)
    acc = np.minimum(
        np.float32(imm2),
        b.reshape(b.shape[0], -1).min(axis=-1, keepdims=True),
    ).astype(np.float32)
    return b, acc


_REGISTERED = {}


def _register_op(name: str, spec: Spec) -> DveOp:
    """Register a custom DVE op at runtime. Tables are generated per-NEFF,
    so this needs no firmware support."""
    if name in _REGISTERED:
        return _REGISTERED[name]
    for op in dve_ops.OPS:
        if op.name == name:
            _REGISTERED[name] = op
            return op
    row = max(dve_ops._SUB_OPCODE_FOR_NAME.values()) + 1
    assert row < 0x20, "no free custom-DVE opcode row"
    dve_ops._SUB_OPCODE_FOR_NAME[name] = row
    shas = {
        ver: DveOpSpec(
            name=name, opcode=row, uops=lower(spec, ver=ver),
            rd1_en=_has_src1(spec),
        ).sha(ver)
        for ver in ("v3", "v4")
    }
    op = DveOp(name, spec, subdim=False, uops_sha=shas)
    dve_ops.OPS.append(op)
    dve_ops.CUSTOM_DVE_SPECS[name] = spec
    _REGISTERED[name] = op
    return op


def _get_ops():
    maxmax = _register_op(
        "TT_MAX_MAX_REDUCE_ANT",
        Spec(body=maxx(Src0, Src1), accum=maxx, accum_init=C0,
             reference=_maxmax_ref),
    )
    distmin = _register_op(
        "DIST2_MIN_REDUCE_ANT",
        Spec(body=sq(Src0 - C0) + sq(Src1 - C1), accum=minn, accum_init=C2,
             reference=_distmin_ref),
    )
    return maxmax, distmin


def _build_program(repeat: int = 1):
    """Build + compile the per-core Bass program. `repeat` re-runs the main
    compute loop inside a hardware For_i for timing (results unchanged)."""
    maxmax, distmin = _get_ops()

    nc = bacc.Bacc("TRN2", target_bir_lowering=False, debug=False)
    pts1 = nc.dram_tensor("pts1", [OBJ_PER_CORE, P, 2], F32, kind="ExternalInput")
    pts2 = nc.dram_tensor("pts2", [OBJ_PER_CORE, P, 2], F32, kind="ExternalInput")
    out = nc.dram_tensor("minsq", [N_GENS, 128, IT], F32, kind="ExternalOutput")

    ones_row = nc.inline_tensor(np.ones((1, P), dtype=np.float32), name="ones_row")
    scr = [nc.dram_tensor(f"scr{s}", [1, P], F32, kind="Internal") for s in range(4)]

    p1 = pts1.ap()
    p2 = pts2.ap()
    o = out.ap()

    # point sets in per-core order: s = 2*obj + (0: set1, 1: set2)
    set_aps = [p1[0], p2[0], p1[1], p2[1]]
    # generations: (query set idx, key set idx)
    gen_sets = [(0, 1), (1, 0), (2, 3), (3, 2)]

    with tile.TileContext(nc) as tc:
        with contextlib.ExitStack() as ctx:
            persist = ctx.enter_context(tc.tile_pool(name="persist", bufs=1))
            temps = ctx.enter_context(tc.tile_pool(name="temps", bufs=2))

            # ---------------- prep: per point set ----------------
            # also used as the discarded elementwise output of the custom DVE
            # ops in the main loop (a stride-0 broadcast out costs ~500ns/op)
            trash = persist.tile([128, JD], F32, tag="trash")
            qrows, krows, sqcol, qcol, krep = [], [], [], [], []
            with tc.tile_pool(name="prep_psum", bufs=2, space="PSUM") as ppsum, \
                 tc.tile_pool(name="bc_psum", bufs=1, space="PSUM") as bpsum:
                from concourse.masks import make_identity
                ident = persist.tile([128, 128], F32, tag="ident")
                make_identity(nc, ident[:])

                for s in range(4):
                    pset = set_aps[s]          # [P, 2] dram AP
                    rows2 = pset.rearrange("p d -> d p")              # [2, P]
                    cols = pset.rearrange("(c p) d -> p c d", p=128)  # [128, IT, 2]

                    # query-role rows [ones; p0; p1] (lhsT), PE pipeline,
                    # replicated at base partitions 0/32/64/96 for 4-way
                    # concurrent row-group matmuls (K=3 << 128)
                    qr = persist.tile([99, P], F32, tag=f"qrows{s}")
                    for rg in (0, 32, 64, 96):
                        nc.sync.dma_start(out=qr[rg:rg + 1, :], in_=ones_row.ap()[:])
                        nc.sync.dma_start(out=qr[rg + 1:rg + 3, :], in_=rows2)
                    qrows.append(qr)

                    # key-role rows [-|p|^2/2; p0; p1] (rhs), PE pipeline
                    kr = persist.tile([99, JP], F32, tag=f"krows{s}")
                    for rg in (0, 32, 64, 96):
                        nc.sync.dma_start(out=kr[rg + 1:rg + 3, :], in_=rows2[:, 0:JP])

                    # key-role replicated coords for the DVE-direct pipeline:
                    # broadcast row -> all 128 partitions via a K=1 ones
                    # matmul (outer product); stride-0 DMA broadcast is ~100x
                    # slower.
                    r0 = persist.tile([128, JD], F32, tag=f"krep0_{s}")
                    r1 = persist.tile([128, JD], F32, tag=f"krep1_{s}")
                    for d, rtile in ((0, r0), (1, r1)):
                        stage = trash
                        nc.sync.dma_start(out=stage[0:1, :], in_=qr[1 + d:2 + d, JP:])
                        bc = bpsum.tile([128, JD], F32, tag="bcast")
                        for k in range(JD // 512):
                            nc.tensor.matmul(
                                bc[:, k * 512:(k + 1) * 512],
                                qr[0:1, 0:128],
                                stage[0:1, k * 512:(k + 1) * 512],
                                start=True, stop=True,
                            )
                        nc.scalar.copy(rtile[:], bc[:])
                    krep.append((r0, r1))

                    # query-role column layout coords + |p|^2
                    c0 = persist.tile([128, IT], F32, tag=f"qcol0_{s}")
                    c1 = persist.tile([128, IT], F32, tag=f"qcol1_{s}")
                    nc.sync.dma_start(out=c0[:], in_=cols[:, :, 0])
                    nc.sync.dma_start(out=c1[:], in_=cols[:, :, 1])
                    qcol.append((c0, c1))
                    m0 = temps.tile([128, IT], F32, tag="m0")
                    sqc = persist.tile([128, IT], F32, tag=f"sqcol{s}")
                    nc.vector.tensor_tensor(m0[:], c0[:], c0[:], op=ALU.mult)
                    nc.vector.tensor_tensor(sqc[:], c1[:], c1[:], op=ALU.mult)
                    nc.vector.tensor_tensor(sqc[:], sqc[:], m0[:], op=ALU.add)
                    sqcol.append(sqc)

                    # -|p|^2/2 row for the PE-pipeline rhs row 0 (keys 0:JP)
                    nsq = temps.tile([128, IT], F32, tag="nsq")
                    nc.vector.tensor_scalar_mul(nsq[:], sqc[:], -0.5)
                    pT = ppsum.tile([IT, 128], F32, tag="pT")
                    nc.tensor.transpose(pT[:], nsq[:], ident[:])
                    pTs = temps.tile([IT, 128], F32, tag="pTs")
                    nc.vector.tensor_copy(out=pTs[:], in_=pT[:])
                    nc.sync.dma_start(
                        out=scr[s].ap()[0].rearrange("(c p) -> c p", p=128),
                        in_=pTs[:],
                    )
                    for rg in (0, 32, 64, 96):
                        nc.sync.dma_start(out=kr[rg:rg + 1, :], in_=scr[s].ap()[:, 0:JP])
                    krows.append(kr)

            # ---------------- main: 4 generations ----------------
            with tc.tile_pool(name="mm_psum", bufs=2, space="PSUM") as mpsum:
                cp_pool = ctx.enter_context(tc.tile_pool(name="cp", bufs=3))

                def emit_itile(g, t, rowmax, dmin):
                    qi, ki = gen_sets[g]
                    lhsT, rhs = qrows[qi], krows[ki]
                    ky0, ky1 = krep[ki]
                    qx0, qx1 = qcol[qi]
                    ps = mpsum.tile([128, JP], F32, tag="ps")
                    for k in range(JP // 512):
                        rg = 32 * k
                        nc.tensor.matmul(
                            ps[:, k * 512:(k + 1) * 512],
                            lhsT[rg:rg + 3, t * 128:(t + 1) * 128],
                            rhs[rg:rg + 3, k * 512:(k + 1) * 512],
                            start=True, stop=True,
                            tile_position=(rg, 0),
                        )
                    # ScalarE drains the whole PSUM tile: banks recycle after
                    # the 2us copy instead of after the DVE fold, and the DVE
                    # fold runs SBUF x SBUF (lower init overhead than PSUM).
                    cp = cp_pool.tile([128, JP], F32, tag="cp")
                    nc.scalar.copy(cp[:], ps[:])
                    dm = nc.vector._custom_dve(
                        distmin,
                        out=trash[:],
                        in0=ky0[:],
                        in1=ky1[:],
                        s0=qx0[:, t:t + 1],
                        s1=qx1[:, t:t + 1],
                        imm2=POS_BIG,
                        accum_out=dmin[:, t:t + 1],
                    )
                    mx = nc.vector._custom_dve(
                        maxmax,
                        out=trash[:, 0:JP // 2],
                        in0=cp[:, 0:JP // 2],
                        in1=cp[:, JP // 2:],
                        s0=NEG_BIG,
                        accum_out=rowmax[:, t:t + 1],
                    )
                    # Alternate distmin/maxmax in DVE program order (the
                    # scheduler would otherwise front-load every distmin and
                    # starve the PE on PSUM). distmin-first: while it runs,
                    # the matmuls + ScalarE copy for this i-tile finish, so
                    # the fold never stalls on the ACT chain.
                    bass._add_dep_helper(
                        mx.ins, dm.ins, sync=False,
                        reason="interleave DVE pipelines",
                    )

                def finish_gen(g, rowmax, dmin):
                    qi, _ = gen_sets[g]
                    # min_j dist^2 = min(|q|^2 - 2*rowmax, dmin)
                    pemin = temps.tile([128, IT], F32, tag="pemin")
                    nc.vector.scalar_tensor_tensor(
                        out=pemin[:],
                        in0=rowmax[:], scalar=-2.0, in1=sqcol[qi][:],
                        op0=ALU.mult, op1=ALU.add,
                    )
                    minsq = temps.tile([128, IT], F32, tag="minsq")
                    nc.vector.tensor_tensor(minsq[:], pemin[:], dmin[:], op=ALU.min)
                    nc.sync.dma_start(out=o[g], in_=minsq[:])

                def body(_iv=None):
                    # interleave two independent generations per loop to give
                    # the scheduler parallel dependency chains
                    for ga, gb in ((0, 2), (1, 3)):
                        accs = {}
                        for g in (ga, gb):
                            rowmax = persist.tile([128, IT], F32, tag=f"rowmax{g}")
                            dmin = persist.tile([128, IT], F32, tag=f"dmin{g}")
                            accs[g] = (rowmax, dmin)
                        for t in range(IT):
                            for g in (ga, gb):
                                emit_itile(g, t, *accs[g])
                        for g in (ga, gb):
                            finish_gen(g, *accs[g])

                if repeat == 1:
                    body()
                else:
                    with tc.For_i(0, repeat, 1):
                        body()

    nc.compile()
    return nc


_CACHE = {}
LAST_RESULTS = None


def _get_program(repeat: int = 1):
    key = ("nc", repeat)
    if key not in _CACHE:
        _CACHE[key] = _build_program(repeat)
    return _CACHE[key]


def kernel(point_set_1: np.ndarray, point_set_2: np.ndarray,
           _trace: bool = False, _repeat: int = 1) -> np.ndarray:
    global LAST_RESULTS
    point_set_1 = np.ascontiguousarray(point_set_1, dtype=np.float32)
    point_set_2 = np.ascontiguousarray(point_set_2, dtype=np.float32)
    assert point_set_1.shape == (N_OBJ, P, 2) and point_set_2.shape == (N_OBJ, P, 2)

    nc = _get_program(_repeat)
    in_maps = []
    for c in range(N_CORES):
        sl = slice(c * OBJ_PER_CORE, (c + 1) * OBJ_PER_CORE)
        in_maps.append({
            "pts1": np.ascontiguousarray(point_set_1[sl]),
            "pts2": np.ascontiguousarray(point_set_2[sl]),
        })
    res = run_bass_kernel_spmd(
        nc, in_maps, core_ids=list(range(N_CORES)), trace=_trace,
    )
    LAST_RESULTS = res

    # host finish: minima -> sqrt -> means -> mask -> final mean
    costs = np.zeros(N_OBJ, dtype=np.float64)
    for c in range(N_CORES):
        minsq = res.results[c]["minsq"]          # [4, 128, IT]
        for obj in range(OBJ_PER_CORE):
            n = c * OBJ_PER_CORE + obj
            d_sum = 0.0
            for direction in range(2):
                g = 2 * obj + direction
                ms = minsq[g].T.reshape(P)       # i = t*128 + m
                d = np.sqrt(np.maximum(ms.astype(np.float64), EPS))
                d_sum += d.mean()
            costs[n] = 0.5 * d_sum
    mask = (point_set_2.reshape(N_OBJ, -1).sum(axis=1, dtype=np.float32) >= 0)
    loss = (costs * mask).sum() / N_OBJ
    return np.asarray(loss, dtype=np.float32)



# revision 3
# speedup vs baseline: 1.1473x; 1.1473x over previous
"""Chamfer 2D loss kernel for Trainium2 (8 NeuronCores, SPMD).

Problem: N=16 objects, two point sets [16, 4096, 2] fp32 each.
Per object: C[i,j] = sqrt(clip(|x_i|^2 + |y_j|^2 - 2 x_i.y_j, 1e-12));
loss = mean_n mask_n * 0.5 * (mean_i min_j C + mean_j min_i C).

Sharding: data-parallel over objects, 2 objects per core; per core 4
"generations" (2 objects x 2 directions), each computing the 4096
per-query min squared distances for query set Q vs key set K.

Per generation, per i-tile of 128 queries, ALL 4096 keys go through the
PE: out[i,j] = q_i.k_j - |k_j|^2/2 via K=3 fp32 matmuls (row-group
tile_position packing) into four 1024-key PSUM tiles (2 banks each,
4-buf pool = all 8 banks).  min_j dist^2 = |q|^2 - 2*max_j out.
The max-reduce is split so DVE and ScalarE stream concurrently:
  - tiles N1,N2 are drained PSUM->SBUF by ScalarE (1 elem/cyc @1.2GHz);
  - tiles D1,D2 are read DIRECTLY from PSUM by the custom DVE fold
    (port0=PSUM, port1=SBUF), so each fold op covers 2048 keys in 1024
    DVE cycles and ScalarE only touches half the keys.
Two fold ops per i-tile accumulate row maxima into separate columns
(rA, rB); a tiny per-generation epilogue combines them.
min over j of sqrt == sqrt of min over j (monotonic), so only the 4096
per-query minima need sqrt, done on host with the means and the mask.
"""

import contextlib

import numpy as np

import concourse.bacc as bacc
import concourse.bass as bass
import concourse.tile as tile
from concourse import mybir
import concourse.dve_ops as dve_ops
from concourse.dve_ops import DveOp
from concourse.dve_spec import (
    Spec, Src0, Src1, C0, maxx, lower, _has_src1,
)
from concourse.dve_uop import DveOpSpec
from concourse.bass_utils import run_bass_kernel_spmd

F32 = mybir.dt.float32
ALU = mybir.AluOpType

N_CORES = 8
N_OBJ = 16
P = 4096          # points per set
OBJ_PER_CORE = N_OBJ // N_CORES      # 2
N_GENS = 2 * OBJ_PER_CORE            # 4 generations per core
IT = P // 128                        # 32 i-tiles of 128 queries
CH = 1024                            # keys per PSUM chunk (2 banks)
EPS = 1e-12
NEG_BIG = -3.0e38


def _maxmax_ref(in0, in1, s0, s1, imm2):
    b = np.maximum(in0, in1).astype(np.float32)
    acc = np.maximum(
        np.asarray(s0, np.float32),
        b.reshape(b.shape[0], -1).max(axis=-1, keepdims=True),
    ).astype(np.float32)
    return b, acc


_REGISTERED = {}


def _register_op(name: str, spec: Spec) -> DveOp:
    """Register a custom DVE op at runtime. Tables are generated per-NEFF,
    so this needs no firmware support."""
    if name in _REGISTERED:
        return _REGISTERED[name]
    for op in dve_ops.OPS:
        if op.name == name:
            _REGISTERED[name] = op
            return op
    row = max(dve_ops._SUB_OPCODE_FOR_NAME.values()) + 1
    assert row < 0x20, "no free custom-DVE opcode row"
    dve_ops._SUB_OPCODE_FOR_NAME[name] = row
    shas = {
        ver: DveOpSpec(
            name=name, opcode=row, uops=lower(spec, ver=ver),
            rd1_en=_has_src1(spec),
        ).sha(ver)
        for ver in ("v3", "v4")
    }
    op = DveOp(name, spec, subdim=False, uops_sha=shas)
    dve_ops.OPS.append(op)
    dve_ops.CUSTOM_DVE_SPECS[name] = spec
    _REGISTERED[name] = op
    return op


def _get_ops():
    maxmax = _register_op(
        "TT_MAX_MAX_REDUCE_ANT",
        Spec(body=maxx(Src0, Src1), accum=maxx, accum_init=C0,
             reference=_maxmax_ref),
    )
    return maxmax


def _build_program(repeat: int = 1):
    """Build + compile the per-core Bass program. `repeat` re-runs the main
    compute loop inside a hardware For_i for timing (results unchanged)."""
    maxmax = _get_ops()

    nc = bacc.Bacc("TRN2", target_bir_lowering=False, debug=False)
    pts1 = nc.dram_tensor("pts1", [OBJ_PER_CORE, P, 2], F32, kind="ExternalInput")
    pts2 = nc.dram_tensor("pts2", [OBJ_PER_CORE, P, 2], F32, kind="ExternalInput")
    out = nc.dram_tensor("minsq", [N_GENS, 128, IT], F32, kind="ExternalOutput")

    ones_row = nc.inline_tensor(np.ones((1, P), dtype=np.float32), name="ones_row")
    scr = [nc.dram_tensor(f"scr{s}", [1, P], F32, kind="Internal") for s in range(4)]

    p1 = pts1.ap()
    p2 = pts2.ap()
    o = out.ap()

    # point sets in per-core order: s = 2*obj + (0: set1, 1: set2)
    set_aps = [p1[0], p2[0], p1[1], p2[1]]
    # generations: (query set idx, key set idx)
    gen_sets = [(0, 1), (1, 0), (2, 3), (3, 2)]

    with tile.TileContext(nc) as tc:
        with contextlib.ExitStack() as ctx:
            persist = ctx.enter_context(tc.tile_pool(name="persist", bufs=1))
            temps = ctx.enter_context(tc.tile_pool(name="temps", bufs=2))

            # discarded elementwise output of the custom DVE fold ops
            trash = persist.tile([128, CH], F32, tag="trash")
            qrows, krows, sqcol = [], [], []
            with tc.tile_pool(name="prep_psum", bufs=2, space="PSUM") as ppsum:
                from concourse.masks import make_identity
                ident = persist.tile([128, 128], F32, tag="ident")
                make_identity(nc, ident[:])

                for s in range(4):
                    pset = set_aps[s]          # [P, 2] dram AP
                    rows2 = pset.rearrange("p d -> d p")              # [2, P]
                    cols = pset.rearrange("(c p) d -> p c d", p=128)  # [128, IT, 2]

                    # query-role rows [ones; p0; p1] (lhsT), replicated at
                    # base partitions 0/32/64/96 for 4-way concurrent
                    # row-group matmuls (K=3 << 128)
                    qr = persist.tile([99, P], F32, tag=f"qrows{s}")
                    for rg in (0, 32, 64, 96):
                        nc.sync.dma_start(out=qr[rg:rg + 1, :], in_=ones_row.ap()[:])
                        nc.sync.dma_start(out=qr[rg + 1:rg + 3, :], in_=rows2)
                    qrows.append(qr)

                    # key-role rows [-|p|^2/2; p0; p1] (rhs), full 4096 keys
                    kr = persist.tile([99, P], F32, tag=f"krows{s}")
                    for rg in (0, 32, 64, 96):
                        nc.sync.dma_start(out=kr[rg + 1:rg + 3, :], in_=rows2)

                    # query-role column layout coords -> |p|^2 per query col
                    c0 = temps.tile([128, IT], F32, tag="qc0")
                    c1 = temps.tile([128, IT], F32, tag="qc1")
                    nc.sync.dma_start(out=c0[:], in_=cols[:, :, 0])
                    nc.sync.dma_start(out=c1[:], in_=cols[:, :, 1])
                    m0 = temps.tile([128, IT], F32, tag="m0")
                    sqc = persist.tile([128, IT], F32, tag=f"sqcol{s}")
                    nc.vector.tensor_tensor(m0[:], c0[:], c0[:], op=ALU.mult)
                    nc.vector.tensor_tensor(sqc[:], c1[:], c1[:], op=ALU.mult)
                    nc.vector.tensor_tensor(sqc[:], sqc[:], m0[:], op=ALU.add)
                    sqcol.append(sqc)

                    # -|p|^2/2 row for the rhs row 0: column->row transpose
                    # via PE identity, bounced through dram scratch
                    nsq = temps.tile([128, IT], F32, tag="nsq")
                    nc.vector.tensor_scalar_mul(nsq[:], sqc[:], -0.5)
                    pT = ppsum.tile([IT, 128], F32, tag="pT")
                    nc.tensor.transpose(pT[:], nsq[:], ident[:])
                    pTs = temps.tile([IT, 128], F32, tag="pTs")
                    nc.vector.tensor_copy(out=pTs[:], in_=pT[:])
                    nc.sync.dma_start(
                        out=scr[s].ap()[0].rearrange("(c p) -> c p", p=128),
                        in_=pTs[:],
                    )
                    for rg in (0, 32, 64, 96):
                        nc.sync.dma_start(out=kr[rg:rg + 1, :], in_=scr[s].ap()[:])
                    krows.append(kr)

            # ---------------- main: 4 generations ----------------
            with tc.tile_pool(name="mm_psum", bufs=4, space="PSUM") as mpsum:
                cp_pool = ctx.enter_context(tc.tile_pool(name="cp", bufs=3))

                def emit_itile(g, t, rA, rB):
                    qi, ki = gen_sets[g]
                    lhsT, rhs = qrows[qi], krows[ki]
                    # PSUM chunks: N1,N2 drained by ScalarE; D1,D2 read
                    # directly by the DVE fold. Alloc order matters: it maps
                    # chunks onto pool buffers so PE(t+1) reuses the banks
                    # freed earliest (N* by ScalarE, D* by the fold).
                    chunks = []
                    for c in range(4):
                        ps = mpsum.tile([128, CH], F32, tag="ps")
                        chunks.append(ps)
                    order = (2, 3, 0, 1)  # key-chunk index of N1,N2,D1,D2
                    for buf, kc in enumerate(order):
                        ps = chunks[buf]
                        for h in range(2):
                            k = 2 * kc + h        # 512-col matmul index 0..7
                            rg = 32 * (k % 4)
                            j0 = k * 512
                            nc.tensor.matmul(
                                ps[:, h * 512:(h + 1) * 512],
                                lhsT[rg:rg + 3, t * 128:(t + 1) * 128],
                                rhs[rg:rg + 3, j0:j0 + 512],
                                start=True, stop=True,
                                tile_position=(rg, 0),
                            )
                    n1, n2, d1, d2 = chunks
                    cp1 = cp_pool.tile([128, CH], F32, tag="cp1")
                    cp2 = cp_pool.tile([128, CH], F32, tag="cp2")
                    nc.scalar.copy(cp1[:], n1[:])
                    nc.scalar.copy(cp2[:], n2[:])
                    nc.vector._custom_dve(
                        maxmax, out=trash[:],
                        in0=d1[:], in1=cp1[:],
                        s0=NEG_BIG, accum_out=rA[:, t:t + 1],
                    )
                    nc.vector._custom_dve(
                        maxmax, out=trash[:],
                        in0=d2[:], in1=cp2[:],
                        s0=NEG_BIG, accum_out=rB[:, t:t + 1],
                    )

                def finish_gen(g, rA, rB):
                    qi, _ = gen_sets[g]
                    # min_j dist^2 = |q|^2 - 2*max(rA, rB)
                    rm = temps.tile([128, IT], F32, tag="rm")
                    nc.vector.tensor_tensor(rm[:], rA[:], rB[:], op=ALU.max)
                    minsq = temps.tile([128, IT], F32, tag="minsq")
                    nc.vector.scalar_tensor_tensor(
                        out=minsq[:],
                        in0=rm[:], scalar=-2.0, in1=sqcol[qi][:],
                        op0=ALU.mult, op1=ALU.add,
                    )
                    nc.sync.dma_start(out=o[g], in_=minsq[:])

                def body(_iv=None):
                    for g in range(N_GENS):
                        rA = persist.tile([128, IT], F32, tag=f"rA{g}")
                        rB = persist.tile([128, IT], F32, tag=f"rB{g}")
                        for t in range(IT):
                            emit_itile(g, t, rA, rB)
                        finish_gen(g, rA, rB)

                if repeat == 1:
                    body()
                else:
                    with tc.For_i(0, repeat, 1):
                        body()

    nc.compile()
    return nc


_CACHE = {}
LAST_RESULTS = None


def _get_program(repeat: int = 1):
    key = ("nc", repeat)
    if key not in _CACHE:
        _CACHE[key] = _build_program(repeat)
    return _CACHE[key]


def kernel(point_set_1: np.ndarray, point_set_2: np.ndarray,
           _trace: bool = False, _repeat: int = 1) -> np.ndarray:
    global LAST_RESULTS
    point_set_1 = np.ascontiguousarray(point_set_1, dtype=np.float32)
    point_set_2 = np.ascontiguousarray(point_set_2, dtype=np.float32)
    assert point_set_1.shape == (N_OBJ, P, 2) and point_set_2.shape == (N_OBJ, P, 2)

    nc = _get_program(_repeat)
    in_maps = []
    for c in range(N_CORES):
        sl = slice(c * OBJ_PER_CORE, (c + 1) * OBJ_PER_CORE)
        in_maps.append({
            "pts1": np.ascontiguousarray(point_set_1[sl]),
            "pts2": np.ascontiguousarray(point_set_2[sl]),
        })
    res = run_bass_kernel_spmd(
        nc, in_maps, core_ids=list(range(N_CORES)), trace=_trace,
    )
    LAST_RESULTS = res

    # host finish: minima -> sqrt -> means -> mask -> final mean
    costs = np.zeros(N_OBJ, dtype=np.float64)
    for c in range(N_CORES):
        minsq = res.results[c]["minsq"]          # [4, 128, IT]
        for obj in range(OBJ_PER_CORE):
            n = c * OBJ_PER_CORE + obj
            d_sum = 0.0
            for direction in range(2):
                g = 2 * obj + direction
                ms = minsq[g].T.reshape(P)       # i = t*128 + m
                d = np.sqrt(np.maximum(ms.astype(np.float64), EPS))
                d_sum += d.mean()
            costs[n] = 0.5 * d_sum
    mask = (point_set_2.reshape(N_OBJ, -1).sum(axis=1, dtype=np.float32) >= 0)
    loss = (costs * mask).sum() / N_OBJ
    return np.asarray(loss, dtype=np.float32)


# revision 16
# speedup vs baseline: 1.6198x; 1.4118x over previous
"""Chamfer 2D loss kernel for Trainium2 (8 NeuronCores, SPMD).

Problem: N=16 objects, two point sets [16, 4096, 2] fp32 each.
Per object: C[i,j] = sqrt(clip(|x_i|^2 + |y_j|^2 - 2 x_i.y_j, 1e-12));
loss = mean_n mask_n * 0.5 * (mean_i min_j C + mean_j min_i C).

Sharding: data-parallel over objects, 2 objects per core; per core 4
"generations" (2 objects x 2 directions), each computing the 4096
per-query min squared distances for query set Q vs key set K.

Per generation, per i-tile of 128 queries, ALL 4096 keys go through the
PE as a K=10 fp16 matmul that computes -d^2/2 = q.k - |k|^2/2 - |q|^2/2
directly in PSUM.  Full fp32 inputs are split hi/lo into fp16 pairs
(k = kh + kl), keeping ~17 mantissa bits through the product terms:
  r0 (1,    mh_k) r1 (1,    ml_k)      m = -|k|^2/2 split
  r2 (q0h, k0h) r3 (q0l, k0h) r4 (q0h, k0l)
  r5 (q1h, k1h) r6 (q1l, k1h) r7 (q1h, k1l)
  r8 (sqh_q, 1) r9 (sql_q, 1)          sq = -|q|^2/2 split
fp16 matmuls stream 1 column/cycle (vs 4 for fp32), so the PE is far off
the critical path.  Four 1024-key PSUM chunks per i-tile (2 banks each,
4-buf pool = all 8 banks): chunks N1,N2 are drained PSUM->SBUF by
ScalarE; chunks D1,D2 are read directly from PSUM by the custom DVE
max-fold (port0=PSUM, port1=SBUF, 2 keys/cycle), accumulating row maxima
of -d^2/2 into per-i-tile columns.  min_j d^2 = -2 * max.
min over j of sqrt == sqrt of min over j (monotonic), so only the 4096
per-query minima need sqrt, done on host with the means and the mask.
"""

import contextlib

import numpy as np

import concourse.bacc as bacc
import concourse.bass as bass
import concourse.tile as tile
from concourse import mybir
import concourse.dve_ops as dve_ops
from concourse.dve_ops import DveOp
from concourse.dve_spec import (
    Spec, Src0, Src1, C0, maxx, lower, _has_src1,
)
from concourse.dve_uop import DveOpSpec
from concourse.bass_utils import run_bass_kernel_spmd

F32 = mybir.dt.float32
F16 = mybir.dt.float16
ALU = mybir.AluOpType

N_CORES = 8
N_OBJ = 16
P = 4096          # points per set
OBJ_PER_CORE = N_OBJ // N_CORES      # 2
N_GENS = 2 * OBJ_PER_CORE            # 4 generations per core
IT = P // 128                        # 32 i-tiles of 128 queries
CH = 1024                            # keys per PSUM chunk (2 banks)
KR = 10                              # matmul contraction rows per group
EPS = 1e-12
NEG_BIG = -3.0e38


def _maxmax_ref(in0, in1, s0, s1, imm2):
    b = np.maximum(in0, in1).astype(np.float32)
    acc = np.maximum(
        np.asarray(s0, np.float32),
        b.reshape(b.shape[0], -1).max(axis=-1, keepdims=True),
    ).astype(np.float32)
    return b, acc


_REGISTERED = {}


def _register_op(name: str, spec: Spec) -> DveOp:
    """Register a custom DVE op at runtime. Tables are generated per-NEFF,
    so this needs no firmware support."""
    if name in _REGISTERED:
        return _REGISTERED[name]
    for op in dve_ops.OPS:
        if op.name == name:
            _REGISTERED[name] = op
            return op
    row = max(dve_ops._SUB_OPCODE_FOR_NAME.values()) + 1
    assert row < 0x20, "no free custom-DVE opcode row"
    dve_ops._SUB_OPCODE_FOR_NAME[name] = row
    shas = {
        ver: DveOpSpec(
            name=name, opcode=row, uops=lower(spec, ver=ver),
            rd1_en=_has_src1(spec),
        ).sha(ver)
        for ver in ("v3", "v4")
    }
    op = DveOp(name, spec, subdim=False, uops_sha=shas)
    dve_ops.OPS.append(op)
    dve_ops.CUSTOM_DVE_SPECS[name] = spec
    _REGISTERED[name] = op
    return op


def _get_ops():
    maxmax = _register_op(
        "TT_MAX_MAX_REDUCE_ANT",
        Spec(body=maxx(Src0, Src1), accum=maxx, accum_init=C0,
             reference=_maxmax_ref),
    )
    return maxmax


def _build_program(repeat: int = 1):
    """Build + compile the per-core Bass program. `repeat` re-runs the main
    compute loop inside a hardware For_i for timing (results unchanged)."""
    maxmax = _get_ops()

    nc = bacc.Bacc("TRN2", target_bir_lowering=False, debug=False)
    pts1 = nc.dram_tensor("pts1", [OBJ_PER_CORE, P, 2], F32, kind="ExternalInput")
    pts2 = nc.dram_tensor("pts2", [OBJ_PER_CORE, P, 2], F32, kind="ExternalInput")
    out = nc.dram_tensor("minsq", [N_GENS, 128, IT], F32, kind="ExternalOutput")

    ones_row = nc.inline_tensor(np.ones((1, P), dtype=np.float32), name="ones_row")
    scr = [nc.dram_tensor(f"scr{s}", [1, P], F32, kind="Internal") for s in range(4)]

    p1 = pts1.ap()
    p2 = pts2.ap()
    o = out.ap()

    # point sets in per-core order: s = 2*obj + (0: set1, 1: set2)
    set_aps = [p1[0], p2[0], p1[1], p2[1]]
    # generations: (query set idx, key set idx)
    gen_sets = [(0, 1), (1, 0), (2, 3), (3, 2)]

    with tile.TileContext(nc) as tc:
        with contextlib.ExitStack() as ctx:
            persist = ctx.enter_context(tc.tile_pool(name="persist", bufs=1))
            temps = ctx.enter_context(tc.tile_pool(name="temps", bufs=2))

            # discarded elementwise output of the custom DVE fold ops
            trash = persist.tile([128, CH], F32, tag="trash")
            qrows, krows = [], []
            with tc.tile_pool(name="prep_psum", bufs=2, space="PSUM") as ppsum, \
                 tc.tile_pool(name="prep", bufs=1) as prep:
                from concourse.masks import make_identity
                ident = prep.tile([128, 128], F32, tag="ident")
                make_identity(nc, ident[:])
                ones_bf = prep.tile([1, P], F16, tag="ones_bf")
                ones_sb = prep.tile([1, P], F32, tag="ones_sb")
                nc.sync.dma_start(out=ones_sb[:], in_=ones_row.ap()[:])
                nc.vector.tensor_copy(out=ones_bf[:], in_=ones_sb[:])

                def hi_lo(src_ap, n_part, tag):
                    """Split fp32 rows [n, P] into fp16 hi + lo tiles."""
                    hi = prep.tile([n_part, P], F16, tag=f"{tag}_hi", name="hi")
                    lo = prep.tile([n_part, P], F16, tag=f"{tag}_lo", name="lo")
                    hi32 = prep.tile([n_part, P], F32, tag="hl32", name="hi32")
                    nc.vector.tensor_copy(out=hi[:], in_=src_ap)
                    nc.vector.tensor_copy(out=hi32[:], in_=hi[:])
                    nc.vector.tensor_tensor(hi32[:], src_ap, hi32[:],
                                            op=ALU.subtract)
                    nc.vector.tensor_copy(out=lo[:], in_=hi32[:])
                    return hi, lo

                for s in range(4):
                    pset = set_aps[s]          # [P, 2] dram AP
                    rows2 = pset.rearrange("p d -> d p")              # [2, P]
                    cols = pset.rearrange("(c p) d -> p c d", p=128)  # [128, IT, 2]

                    # coordinate rows (fp32) -> fp16 hi/lo [2, P]
                    crows = prep.tile([2, P], F32, tag="crows")
                    nc.sync.dma_start(out=crows[:], in_=rows2)
                    chi, clo = hi_lo(crows[:], 2, "c")

                    # -|p|^2/2 row: build via column-wise square/sum, PE
                    # transpose, dram bounce -> [1, P] fp32 -> fp16 hi/lo
                    c0 = prep.tile([128, IT], F32, tag="qc0")
                    c1 = prep.tile([128, IT], F32, tag="qc1")
                    nc.sync.dma_start(out=c0[:], in_=cols[:, :, 0])
                    nc.sync.dma_start(out=c1[:], in_=cols[:, :, 1])
                    m0 = prep.tile([128, IT], F32, tag="m0")
                    sqc = prep.tile([128, IT], F32, tag="sqc")
                    nc.vector.tensor_tensor(m0[:], c0[:], c0[:], op=ALU.mult)
                    nc.vector.tensor_tensor(sqc[:], c1[:], c1[:], op=ALU.mult)
                    nc.vector.tensor_tensor(sqc[:], sqc[:], m0[:], op=ALU.add)
                    nsq = prep.tile([128, IT], F32, tag="nsq")
                    nc.vector.tensor_scalar_mul(nsq[:], sqc[:], -0.5)
                    pT = ppsum.tile([IT, 128], F32, tag="pT")
                    nc.tensor.transpose(pT[:], nsq[:], ident[:])
                    pTs = prep.tile([IT, 128], F32, tag="pTs")
                    nc.vector.tensor_copy(out=pTs[:], in_=pT[:])
                    nc.sync.dma_start(
                        out=scr[s].ap()[0].rearrange("(c p) -> c p", p=128),
                        in_=pTs[:],
                    )
                    srow = prep.tile([1, P], F32, tag="srow")
                    nc.sync.dma_start(out=srow[:], in_=scr[s].ap()[:])
                    shi, slo = hi_lo(srow[:], 1, "s")

                    # assemble the replicated row-group tiles (see module
                    # docstring for the 10-row pairing)
                    qg = [ones_bf[0:1], ones_bf[0:1], chi[0:1], clo[0:1],
                          chi[0:1], chi[1:2], clo[1:2], chi[1:2],
                          shi[0:1], slo[0:1]]
                    kg = [shi[0:1], slo[0:1], chi[0:1], chi[0:1], clo[0:1],
                          chi[1:2], chi[1:2], clo[1:2],
                          ones_bf[0:1], ones_bf[0:1]]
                    qr = persist.tile([96 + KR, P], F16, tag=f"qrows{s}")
                    kr = persist.tile([96 + KR, P], F16, tag=f"krows{s}")
                    for rg in (0, 32, 64, 96):
                        for r in range(KR):
                            nc.sync.dma_start(out=qr[rg + r:rg + r + 1, :],
                                              in_=qg[r])
                            nc.sync.dma_start(out=kr[rg + r:rg + r + 1, :],
                                              in_=kg[r])
                    qrows.append(qr)
                    krows.append(kr)

            # ---------------- main: 4 generations ----------------
            with tc.tile_pool(name="mm_psum", bufs=4, space="PSUM") as mpsum:
                cp_pool = ctx.enter_context(tc.tile_pool(name="cp", bufs=3))

                def emit_itile(g, t, rA, rB):
                    qi, ki = gen_sets[g]
                    lhsT, rhs = qrows[qi], krows[ki]
                    # PSUM chunks: buffers 0,1 = N1,N2 (ScalarE-drained),
                    # 2,3 = D1,D2 (DVE reads PSUM directly). Alloc order maps
                    # chunks onto pool buffers so PE(t+1) reuses banks freed
                    # earliest (N* by ScalarE, D* by the fold).
                    chunks = [mpsum.tile([128, CH], F32, tag="ps", name="ps")
                              for _ in range(4)]
                    order = (2, 3, 0, 1)  # key-chunk index of N1,N2,D1,D2
                    for buf, kc in enumerate(order):
                        ps = chunks[buf]
                        for h in range(2):
                            k = 2 * kc + h        # 512-col matmul index 0..7
                            rg = 32 * (k % 4)
                            j0 = k * 512
                            nc.tensor.matmul(
                                ps[:, h * 512:(h + 1) * 512],
                                lhsT[rg:rg + KR, t * 128:(t + 1) * 128],
                                rhs[rg:rg + KR, j0:j0 + 512],
                                start=True, stop=True,
                                tile_position=(rg, 0),
                            )
                    n1, n2, d1, d2 = chunks
                    cp1 = cp_pool.tile([128, CH], F32, tag="cp1")
                    cp2 = cp_pool.tile([128, CH], F32, tag="cp2")
                    nc.scalar.copy(cp1[:], n1[:])
                    nc.scalar.copy(cp2[:], n2[:])
                    nc.vector._custom_dve(
                        maxmax, out=trash[:],
                        in0=d1[:], in1=cp1[:],
                        s0=NEG_BIG, accum_out=rA[:, t:t + 1],
                    )
                    nc.vector._custom_dve(
                        maxmax, out=trash[:],
                        in0=d2[:], in1=cp2[:],
                        s0=NEG_BIG, accum_out=rB[:, t:t + 1],
                    )

                def finish_gen(g, rA, rB):
                    # PSUM held -d^2/2; min_j d^2 = -2 * max
                    rm = temps.tile([128, IT], F32, tag="rm")
                    nc.vector.tensor_tensor(rm[:], rA[:], rB[:], op=ALU.max)
                    minsq = temps.tile([128, IT], F32, tag="minsq")
                    nc.vector.tensor_scalar_mul(minsq[:], rm[:], -2.0)
                    nc.sync.dma_start(out=o[g], in_=minsq[:])

                def body(_iv=None):
                    for g in range(N_GENS):
                        rA = persist.tile([128, IT], F32, tag=f"rA{g}")
                        rB = persist.tile([128, IT], F32, tag=f"rB{g}")
                        for t in range(IT):
                            emit_itile(g, t, rA, rB)
                        finish_gen(g, rA, rB)

                if repeat == 1:
                    body()
                else:
                    with tc.For_i(0, repeat, 1):
                        body()

    nc.compile()
    return nc


_CACHE = {}
LAST_RESULTS = None


def _get_program(repeat: int = 1):
    key = ("nc", repeat)
    if key not in _CACHE:
        _CACHE[key] = _build_program(repeat)
    return _CACHE[key]


def kernel(point_set_1: np.ndarray, point_set_2: np.ndarray,
           _trace: bool = False, _repeat: int = 1) -> np.ndarray:
    global LAST_RESULTS
    point_set_1 = np.ascontiguousarray(point_set_1, dtype=np.float32)
    point_set_2 = np.ascontiguousarray(point_set_2, dtype=np.float32)
    assert point_set_1.shape == (N_OBJ, P, 2) and point_set_2.shape == (N_OBJ, P, 2)

    nc = _get_program(_repeat)
    in_maps = []
    for c in range(N_CORES):
        sl = slice(c * OBJ_PER_CORE, (c + 1) * OBJ_PER_CORE)
        in_maps.append({
            "pts1": np.ascontiguousarray(point_set_1[sl]),
            "pts2": np.ascontiguousarray(point_set_2[sl]),
        })
    res = run_bass_kernel_spmd(
        nc, in_maps, core_ids=list(range(N_CORES)), trace=_trace,
    )
    LAST_RESULTS = res

    # host finish: minima -> sqrt -> means -> mask -> final mean
    costs = np.zeros(N_OBJ, dtype=np.float64)
    for c in range(N_CORES):
        minsq = res.results[c]["minsq"]          # [4, 128, IT]
        for obj in range(OBJ_PER_CORE):
            n = c * OBJ_PER_CORE + obj
            d_sum = 0.0
            for direction in range(2):
                g = 2 * obj + direction
                ms = minsq[g].T.reshape(P)       # i = t*128 + m
                d = np.sqrt(np.maximum(ms.astype(np.float64), EPS))
                d_sum += d.mean()
            costs[n] = 0.5 * d_sum
    mask = (point_set_2.reshape(N_OBJ, -1).sum(axis=1, dtype=np.float32) >= 0)
    loss = (costs * mask).sum() / N_OBJ
    return np.asarray(loss, dtype=np.float32)
